# revision 10
# baseline (speedup 1.0000x reference)
"""Trainium2 Bass kernel for VITS-style multi-head attention with windowed
relative position embeddings (window=4), batch 8 x channels 512 x time 1024.

Strategy: pure data parallelism — one batch element per NeuronCore (8 cores).
Per core:
  - Q/K projections in [channel, time] layout; V projected directly transposed
    as [time, channel] (needed as PV stationary operand).
  - Per (head, 128-row tile): scores via PE matmul, relative-key logits R9 =
    Qs @ Krel^T [128, 9] skewed into a banded [128, 136] bias via a
    stride-137-write / stride-136-read DRAM round trip, added to scores in
    PSUM; softmax (exp with free-dim accumulation, no max subtraction — scores
    are ~N(0,1)); P transposed by PE for the PV matmul; the value-side band
    P9[i,d] = P[i, i+d-4] extracted with the inverse skew (write pitch 136,
    read pitch 137) and applied as a [9, KC] matmul.
  - Matmuls use float32r (full-rate fp32 path; moving dims kept >= 256).
"""

import os
import sys

sys.path.insert(0, "/opt/trn_rl_repo")

import numpy as np

import concourse.bass as bass
import concourse.mybir as mybir
import concourse.tile as tile
from concourse import bacc
from concourse.bass_utils import run_bass_kernel_spmd
from concourse.masks import make_identity

B, C, T = 8, 512, 1024
H = 8
KC = C // H  # 64
WINDOW = 4
M_REL = 2 * WINDOW + 1  # 9
SCALE = 1.0 / float(np.sqrt(KC))
N_CORES = 8
NT = T // 128  # 8 row tiles per head
NSCR = 4  # dram skew scratch rotation depth

F32 = mybir.dt.float32
F32R = mybir.dt.float32r
AF = mybir.ActivationFunctionType


def r(ap):
    return ap.bitcast(F32R)


def build_kernel():
    nc = bacc.Bacc(
        "TRN2", target_bir_lowering=False, debug=False, num_devices=N_CORES
    )

    x_d = nc.dram_tensor("x", [C, T], F32, kind="ExternalInput")
    c_d = nc.dram_tensor("c", [C, T], F32, kind="ExternalInput")
    w_d = {
        n: nc.dram_tensor(n, [C, C], F32, kind="ExternalInput")
        for n in ("Wq", "Wk", "Wv", "Wo")
    }
    b_d = {
        n: nc.dram_tensor(n, [C], F32, kind="ExternalInput")
        for n in ("bq", "bk", "bv", "bo")
    }
    ek_d = nc.dram_tensor("emb_rel_k", [1, M_REL, KC], F32, kind="ExternalInput")
    ev_d = nc.dram_tensor("emb_rel_v", [1, M_REL, KC], F32, kind="ExternalInput")
    y_d = nc.dram_tensor("y", [C, T], F32, kind="ExternalOutput")

    # DRAM skew scratches. band: write [128,9] at pitch 137, read [128,136] at
    # pitch 136 (gaps must stay zero). extr: write [128,136] at pitch 136,
    # read [128,9] at pitch 137.
    band_scr = [
        nc.dram_tensor(f"band_scr{i}", [128 * 137], F32, kind="Internal")
        for i in range(NSCR)
    ]
    extr_scr = [
        nc.dram_tensor(f"extr_scr{i}", [128 * 137], F32, kind="Internal")
        for i in range(NSCR)
    ]

    CC = C // 128  # 4 channel chunks

    with tile.TileContext(nc) as tc:
        with (
            tc.tile_pool(name="const", bufs=1) as constp,
            tc.tile_pool(name="wpool", bufs=1) as wpool,
            tc.tile_pool(name="xc", bufs=1) as xcp,
            tc.tile_pool(name="qkv", bufs=1) as qkvp,
            tc.tile_pool(name="outm", bufs=1) as outp,
            tc.tile_pool(name="psb", bufs=2, space="PSUM") as psb,
            tc.tile_pool(name="pss", bufs=4, space="PSUM") as pss,
            tc.tile_pool(name="esb", bufs=3) as esbp,
            tc.tile_pool(name="ptp", bufs=2) as ptp,
            tc.tile_pool(name="bandp", bufs=3) as bandp,
            tc.tile_pool(name="smallp", bufs=4) as smallp,
            tc.tile_pool(name="ysb", bufs=3) as ysbp,
        ):
            # ---- constants / params ----
            ident0 = constp.tile([128, 128], F32, tag="ident0")
            make_identity(nc, ident0[:])
            ident = constp.tile([128, 128], F32, tag="ident")
            nc.gpsimd.tensor_copy(r(ident[:]), ident0[:])

            zrow = constp.tile([128, 137], F32, tag="zrow")
            nc.gpsimd.memset(zrow[:], 0.0)
            for i in range(NSCR):
                nc.sync.dma_start(
                    band_scr[i].rearrange("(r c) -> r c", c=137), zrow[:]
                )

            wsb = {}
            for n in ("Wq", "Wk", "Wv", "Wo"):
                wsb[n] = [wpool.tile([128, C], F32, tag=f"{n}{i}", name=f"{n}_{i}") for i in range(CC)]
                for cc in range(CC):
                    wst = xcp.tile([128, C], F32, tag="wstage", bufs=3, name=f"wst_{n}{cc}")
                    nc.sync.dma_start(
                        wst[:], w_d[n][cc * 128 : (cc + 1) * 128, :]
                    )
                    nc.vector.tensor_copy(r(wsb[n][cc][:]), wst[:])

            # per-partition bias views [128, 4]: col a = channel a*128+p
            bview = {}
            for n in ("bq", "bk", "bo"):
                t = constp.tile([128, CC], F32, tag=n)
                nc.sync.dma_start(t[:], b_d[n].rearrange("(a p) -> p a", p=128))
                bview[n] = t
            bqs = constp.tile([128, CC], F32, tag="bqs")
            nc.vector.tensor_scalar_mul(bqs[:], bview["bq"][:], SCALE)

            bv_row0 = constp.tile([1, C], F32, tag="bv_row0")
            nc.sync.dma_start(bv_row0[:], b_d["bv"][None, :])
            bv_row = constp.tile([1, C], F32, tag="bv_row")
            nc.gpsimd.tensor_copy(r(bv_row[:]), bv_row0[:])
            ones0 = constp.tile([1, 128], F32, tag="ones0")
            nc.gpsimd.memset(ones0[:], 1.0)
            ones1 = constp.tile([1, 128], F32, tag="ones1")
            nc.gpsimd.tensor_copy(r(ones1[:]), ones0[:])

            # krel duplicated in both partition halves so odd heads (base
            # partition 64) can use it as a matmul operand
            krel0 = constp.tile([128, 16], F32, tag="krel0")
            nc.gpsimd.memset(krel0[:], 0.0)
            nc.sync.dma_start(krel0[0:KC, 0:M_REL], ek_d[0].rearrange("m k -> k m"))
            nc.sync.dma_start(krel0[KC:128, 0:M_REL], ek_d[0].rearrange("m k -> k m"))
            krel = constp.tile([128, 16], F32, tag="krel")
            nc.gpsimd.tensor_copy(r(krel[:]), krel0[:])
            vrel0 = constp.tile([M_REL, KC], F32, tag="vrel0")
            nc.sync.dma_start(vrel0[:], ev_d[0])
            vrel = constp.tile([M_REL, KC], F32, tag="vrel")  # [9, 64]
            nc.gpsimd.tensor_copy(r(vrel[:]), vrel0[:])

            x_sb = [xcp.tile([128, T], F32, tag=f"x{i}", name=f"x_sb{i}") for i in range(CC)]
            c_sb = [xcp.tile([128, T], F32, tag=f"c{i}", name=f"c_sb{i}") for i in range(CC)]
            for cc in range(CC):
                xst = xcp.tile([128, T], F32, tag="xstage", bufs=3, name=f"xst{cc}")
                nc.sync.dma_start(xst[:], x_d[cc * 128 : (cc + 1) * 128, :])
                nc.vector.tensor_copy(r(x_sb[cc][:]), xst[:])
                cst = xcp.tile([128, T], F32, tag="xstage", bufs=3, name=f"cst{cc}")
                nc.sync.dma_start(cst[:], c_d[cc * 128 : (cc + 1) * 128, :])
                nc.vector.tensor_copy(r(c_sb[cc][:]), cst[:])

            q_sb = [qkvp.tile([128, T], F32, tag=f"q{i}", name=f"q_sb{i}") for i in range(CC)]
            k_sb = [qkvp.tile([128, T], F32, tag=f"k{i}", name=f"k_sb{i}") for i in range(CC)]
            vt_sb = [qkvp.tile([128, C], F32, tag=f"vt{i}", name=f"vt_sb{i}") for i in range(NT)]
            out_sb = [outp.tile([128, T], F32, tag=f"out{i}", name=f"out_sb{i}") for i in range(CC)]

            # ---- projections: q = (Wq^T x + bq) * scale, k = Wk^T c + bk ----
            for (dst, src, wn, bias, scale) in (
                (q_sb, x_sb, "Wq", bqs, SCALE),
                (k_sb, c_sb, "Wk", bview["bk"], 1.0),
            ):
                for dt in range(CC):
                    for tch in range(2):
                        ps = psb.tile([128, 512], F32, tag="big")
                        for cc in range(CC):
                            nc.tensor.matmul(
                                ps[:],
                                r(wsb[wn][cc][:, dt * 128 : (dt + 1) * 128]),
                                r(src[cc][:, tch * 512 : (tch + 1) * 512]),
                                start=(cc == 0),
                                stop=(cc == CC - 1),
                            )
                        nc.scalar.activation(
                            r(dst[dt][:, tch * 512 : (tch + 1) * 512]),
                            ps[:],
                            AF.Identity,
                            bias=bias[:, dt : dt + 1],
                            scale=scale,
                        )

            # ---- V, produced transposed: vt[t, c] = sum_cc c[cc, t] Wv[cc, c] + bv
            for jt in range(NT):
                ps = psb.tile([128, 512], F32, tag="big")
                for cc in range(CC):
                    nc.tensor.matmul(
                        ps[:],
                        r(c_sb[cc][:, jt * 128 : (jt + 1) * 128]),
                        r(wsb["Wv"][cc][:]),
                        start=(cc == 0),
                        stop=False,
                    )
                nc.tensor.matmul(
                    ps[:], r(ones1[:]), r(bv_row[:]), start=False, stop=True
                )
                nc.scalar.activation(r(vt_sb[jt][:]), ps[:], AF.Copy)

            # ---- attention ----
            uidx = 0
            for h in range(H):
                qt = q_sb[h // 2]
                kt = k_sb[h // 2]
                prow = (h % 2) * KC
                qh = qt[prow : prow + KC, :]  # [64, 1024]
                kh = kt[prow : prow + KC, :]
                for pair in range(NT // 2):
                    i0p = pair * 256
                    pt = ptp.tile([128, 2048], F32, tag="pt")
                    p9t_pair = smallp.tile([M_REL, 256], F32, tag="p9t")
                    for u in range(2):
                        i0 = i0p + u * 128
                        bscr = band_scr[uidx % NSCR]
                        escr = extr_scr[uidx % NSCR]
                        uidx += 1

                        S = psb.tile([128, 1024], F32, tag="big")
                        for jch in range(2):
                            nc.tensor.matmul(
                                S[:, jch * 512 : (jch + 1) * 512],
                                r(qh[:, i0 : i0 + 128]),
                                r(kh[:, jch * 512 : (jch + 1) * 512]),
                                start=True,
                                stop=True,
                            )
                        R9 = pss.tile([128, 256], F32, tag="small")
                        nc.tensor.matmul(
                            R9[:, 0:16],
                            r(qh[:, i0 : i0 + 128]),
                            r(krel[prow : prow + KC, :]),
                            start=True, stop=True,
                        )
                        # skew R9 -> banded bias, add into scores
                        r9s = smallp.tile([128, 16], F32, tag="r9")
                        nc.vector.tensor_copy(r9s[:, 0:M_REL], R9[:, 0:M_REL])
                        nc.sync.dma_start(
                            bscr.rearrange("(r c) -> r c", c=137)[:, 0:M_REL],
                            r9s[:, 0:M_REL],
                        )
                        Bnd = bandp.tile([128, 136], F32, tag="band")
                        nc.sync.dma_start(
                            Bnd[:],
                            bscr[0 : 128 * 136].rearrange("(r c) -> r c", c=136),
                        )
                        lo = max(i0 - 4, 0)
                        hi = min(i0 + 132, T)
                        bl = lo - (i0 - 4)
                        nc.vector.tensor_add(
                            S[:, lo:hi], S[:, lo:hi], Bnd[:, bl : bl + (hi - lo)]
                        )

                        # softmax (no max subtraction; scores ~ N(0,1))
                        E = esbp.tile([128, 1032], F32, tag="e")
                        nc.gpsimd.memset(E[:, 0:4], 0.0)
                        nc.gpsimd.memset(E[:, 1028:1032], 0.0)
                        st = smallp.tile([128, 2], F32, tag="st")
                        nc.scalar.activation(
                            r(E[:, 4:1028]), S[:], AF.Exp, accum_out=st[:, 0:1]
                        )
                        nc.vector.reciprocal(st[:, 1:2], st[:, 0:1])
                        nc.vector.tensor_scalar_mul(
                            r(E[:, 4:1028]), r(E[:, 4:1028]), st[:, 1:2]
                        )

                        # transpose P for the PV matmul
                        ET = psb.tile([128, 1024], F32, tag="big")
                        for jc in range(8):
                            nc.tensor.transpose(
                                r(ET[:, jc * 128 : (jc + 1) * 128]),
                                r(E[:, 4 + jc * 128 : 4 + (jc + 1) * 128]),
                                r(ident[:]),
                            )
                        nc.vector.tensor_copy(
                            r(pt[:, u * 1024 : u * 1024 + 512]), r(ET[:, 0:512])
                        )
                        nc.scalar.copy(
                            r(pt[:, u * 1024 + 512 : u * 1024 + 1024]),
                            r(ET[:, 512:1024]),
                        )

                        # value-side band extraction P9[p,d] = P[i0+p, i0+p+d-4]
                        nc.sync.dma_start(
                            escr[0 : 128 * 136].rearrange("(r c) -> r c", c=136),
                            E[:, i0 : i0 + 136],
                        )
                        p9 = smallp.tile([128, 16], F32, tag="p9")
                        nc.sync.dma_start(
                            p9[:, 0:M_REL],
                            escr.rearrange("(r c) -> r c", c=137)[:, 0:M_REL],
                        )
                        p9r = smallp.tile([128, 16], F32, tag="p9r")
                        nc.gpsimd.tensor_copy(
                            r(p9r[:, 0:M_REL]), p9[:, 0:M_REL]
                        )
                        P9T = pss.tile([128, 256], F32, tag="small")
                        nc.tensor.transpose(
                            r(P9T[0:M_REL, 0:128]), r(p9r[:, 0:M_REL]), r(ident[:])
                        )
                        nc.scalar.copy(
                            r(p9t_pair[:, u * 128 : (u + 1) * 128]),
                            r(P9T[0:M_REL, 0:128]),
                        )

                    # out^T[kc, i] for the pair: PV + relative-value matmuls
                    outT = pss.tile([128, 256], F32, tag="small")
                    ptv = pt[:].rearrange("p (u n) -> p u n", u=2)
                    for jc in range(8):
                        nc.tensor.matmul(
                            outT[0:KC, :],
                            r(vt_sb[jc][:, h * KC : (h + 1) * KC]),
                            r(ptv[:, :, jc * 128 : (jc + 1) * 128]),
                            start=(jc == 0),
                            stop=False,
                        )
                    nc.tensor.matmul(
                        outT[0:KC, :], r(vrel[:]), r(p9t_pair[:]),
                        start=False, stop=True,
                    )
                    nc.scalar.copy(
                        r(out_sb[h // 2][prow : prow + KC, i0p : i0p + 256]),
                        outT[0:KC, :],
                    )

            # ---- output projection: y = Wo^T out + bo ----
            for dt in range(CC):
                for tch in range(2):
                    ps = psb.tile([128, 512], F32, tag="big")
                    for cc in range(CC):
                        nc.tensor.matmul(
                            ps[:],
                            r(wsb["Wo"][cc][:, dt * 128 : (dt + 1) * 128]),
                            r(out_sb[cc][:, tch * 512 : (tch + 1) * 512]),
                            start=(cc == 0),
                            stop=(cc == CC - 1),
                        )
                    yt = ysbp.tile([128, 512], F32, tag="y")
                    nc.scalar.activation(
                        yt[:],
                        ps[:],
                        AF.Identity,
                        bias=bview["bo"][:, dt : dt + 1],
                        scale=1.0,
                    )
                    nc.sync.dma_start(
                        y_d[dt * 128 : (dt + 1) * 128, tch * 512 : (tch + 1) * 512],
                        yt[:],
                    )

    nc.compile()
    return nc


_NC_CACHE = None


def kernel(x, c, Wq, bq, Wk, bk, Wv, bv, Wo, bo, emb_rel_k, emb_rel_v):
    global _NC_CACHE
    if _NC_CACHE is None:
        _NC_CACHE = build_kernel()
    nc = _NC_CACHE

    def f32(a):
        return np.ascontiguousarray(np.asarray(a), dtype=np.float32)

    shared = {
        "Wq": f32(Wq), "bq": f32(bq), "Wk": f32(Wk), "bk": f32(bk),
        "Wv": f32(Wv), "bv": f32(bv), "Wo": f32(Wo), "bo": f32(bo),
        "emb_rel_k": f32(emb_rel_k), "emb_rel_v": f32(emb_rel_v),
    }
    in_maps = [
        {"x": f32(x[b]), "c": f32(c[b]), **shared} for b in range(N_CORES)
    ]
    res = run_bass_kernel_spmd(nc, in_maps, core_ids=list(range(N_CORES)))
    return np.stack([res.results[b]["y"] for b in range(N_CORES)], axis=0)
